# revision 17
# baseline (speedup 1.0000x reference)
"""TRN2 Bass kernel for nn_Attn_63230508532520.

reference:
    proj = history @ W.T + b            # [S1, N]
    energies = out_state @ proj.T       # [S2, S1]
    out = softmax(energies, axis=-1)

Math used here:
    energies = out_state @ W @ history.T + (out_state @ b) 1^T
    The bias term is constant per row -> softmax-invariant -> dropped.
    G = out_state @ W (per-core slice), scores = G @ history.T, row softmax.

Sharding: out_state rows (S2=4096) split across 8 cores (512 rows each);
W and history replicated. Host pre-transposes out_state slices and history
(pure layout choice), all matmul operands fed as float32r (TRN2 rounds to
1s8e11m internally; 4x faster than native fp32 matmul).

Per-core pipeline:
  Phase A: G.T [1024, 512] = W.T-stationary matmuls, accumulated over n.
  Phase B: for each t-block j (8 x 512 cols) stream history.T tiles, compute
           scores [128,512] per s-chunk i into PSUM, take block max (DVE),
           exp(x - blockmax) with per-row accumulation (ACT) into SBUF.
  Phase C: flash-style deferred normalization: global max per row from the 8
           block maxes, rescale factors exp(m_ij - M_i)/S_i, one multiply
           pass, DMA out.
"""
import os
import numpy as np
from contextlib import ExitStack

S2, S1, N = 4096, 4096, 1024
NCORES = 8
SC = S2 // NCORES          # 512 rows per core
NB_T = S1 // 512           # 8 t-blocks
NB_M = N // 128            # 8 contraction chunks
NB_I = SC // 128           # 4 s-chunks per core

_CACHE = {}


def _build():
    import concourse.bacc as bacc
    import concourse.mybir as mybir
    import concourse.tile as tile

    F32 = mybir.dt.float32
    F32R = mybir.dt.float32r

    nc = bacc.Bacc()
    w_r = nc.declare_dram_parameter("w_r", [N, N], F32R, isOutput=False)
    ost_r = nc.declare_dram_parameter("ost_r", [N, SC], F32R, isOutput=False)
    ht_r = nc.declare_dram_parameter("ht_r", [N, S1], F32R, isOutput=False)
    probs = nc.declare_dram_parameter("probs", [SC, S1], mybir.dt.float16, isOutput=True)

    with tile.TileContext(nc) as tc, ExitStack() as ctx:
        gt_pool = ctx.enter_context(tc.tile_pool(name="gt", bufs=1))
        exp_pool = ctx.enter_context(tc.tile_pool(name="exp", bufs=1))
        ht_pool = ctx.enter_context(tc.tile_pool(name="ht", bufs=2))
        small = ctx.enter_context(tc.tile_pool(name="small", bufs=1))
        ps = ctx.enter_context(tc.tile_pool(name="ps", bufs=8, space="PSUM"))

        # ---- Phase A: G.T = (out_state_slice @ W).T, [m, s] layout ----
        gt = []
        with tc.tile_pool(name="win", bufs=1) as win:
            w_sb, ost_sb = [], []
            for n in range(NB_M):
                wt = win.tile([128, N], F32R, tag=f"w{n}")
                nc.sync.dma_start(out=wt, in_=w_r[n * 128:(n + 1) * 128, :])
                w_sb.append(wt)
                ot = win.tile([128, SC], F32R, tag=f"o{n}")
                nc.sync.dma_start(out=ot, in_=ost_r[n * 128:(n + 1) * 128, :])
                ost_sb.append(ot)
            for m in range(NB_M):
                pg = ps.tile([128, SC], F32, tag="ps")
                for n in range(NB_M):
                    nc.tensor.matmul(pg[:], lhsT=w_sb[n][:, m * 128:(m + 1) * 128],
                                     rhs=ost_sb[n][:],
                                     start=(n == 0), stop=(n == NB_M - 1))
                g = gt_pool.tile([128, SC], F32R, tag=f"gt{m}")
                nc.vector.tensor_copy(out=g[:], in_=pg[:])
                gt.append(g)

        # ---- Phase B: scores + streaming exp ----
        out_pool = ctx.enter_context(tc.tile_pool(name="outp", bufs=2))
        expb = [exp_pool.tile([128, S1], mybir.dt.float16, tag=f"exp{i}", name=f"exp{i}") for i in range(NB_I)]
        nmax = [small.tile([128, NB_T], F32, tag=f"nmax{i}", name=f"nmax{i}") for i in range(NB_I)]
        ssum = [small.tile([128, NB_T], F32, tag=f"ssum{i}", name=f"ssum{i}") for i in range(NB_I)]

        def finalize(i):
            """Global max over block maxes, rescale factors, scale+store.

            nmax holds nm_ij = -m_ij; NM_i = min_j nm_ij = -M_i, so
            e_ij = exp(m_ij - M_i) = exp(-nm_ij + NM_i) = Exp(scale=-1, bias=NM_i).
            """
            nm = small.tile([128, 1], F32, tag=f"nm{i}", name=f"nm{i}")
            nc.vector.tensor_reduce(out=nm[:], in_=nmax[i][:],
                                    axis=mybir.AxisListType.X,
                                    op=mybir.AluOpType.min)
            e = small.tile([128, NB_T], F32, tag=f"e{i}", name=f"e{i}")
            nc.scalar.activation(out=e[:], in_=nmax[i][:],
                                 func=mybir.ActivationFunctionType.Exp,
                                 bias=nm[:], scale=-1.0)
            wsum = small.tile([128, NB_T], F32, tag=f"ws{i}", name=f"ws{i}")
            nc.vector.tensor_mul(wsum[:], e[:], ssum[i][:])
            s = small.tile([128, 1], F32, tag=f"s{i}", name=f"s{i}")
            nc.vector.tensor_reduce(out=s[:], in_=wsum[:],
                                    axis=mybir.AxisListType.X,
                                    op=mybir.AluOpType.add)
            r = small.tile([128, 1], F32, tag=f"r{i}", name=f"r{i}")
            nc.vector.reciprocal(out=r[:], in_=s[:])
            f = small.tile([128, NB_T], F32, tag=f"f{i}", name=f"f{i}")
            nc.vector.tensor_scalar_mul(f[:], e[:], r[:])
            o = out_pool.tile([128, S1], mybir.dt.float16,
                              tag=f"out{i % 2}", name=f"out{i}")
            for j in range(NB_T):
                sl = slice(j * 512, (j + 1) * 512)
                if j < 5:
                    nc.vector.tensor_scalar_mul(o[:, sl], expb[i][:, sl],
                                                f[:, j:j + 1])
                else:
                    nc.scalar.mul(o[:, sl], expb[i][:, sl], f[:, j:j + 1])
                if j == 3:
                    nc.sync.dma_start(out=probs[i * 128:(i + 1) * 128, 0:2048],
                                      in_=o[:, 0:2048])
            nc.scalar.dma_start(out=probs[i * 128:(i + 1) * 128, 2048:4096],
                                in_=o[:, 2048:4096])

        ht_quad = None
        for j in range(NB_T):
            if j % 2 == 0:
                ht_quad = []
                for m in range(NB_M):
                    t = ht_pool.tile([128, 1024], F32R, tag=f"ht{m}",
                                     name=f"ht{m}")
                    nc.sync.dma_start(out=t, in_=ht_r[m * 128:(m + 1) * 128,
                                                      j * 512:(j + 2) * 512])
                    ht_quad.append(t)
            off = (j % 2) * 512
            ht_sb = [t[:, off:off + 512] for t in ht_quad]
            for i in range(NB_I):
                pscore = ps.tile([128, 512], F32, tag="ps")
                for m in range(NB_M):
                    nc.tensor.matmul(pscore[:],
                                     lhsT=gt[m][:, i * 128:(i + 1) * 128],
                                     rhs=ht_sb[m][:],
                                     start=(m == 0), stop=(m == NB_M - 1))
                nc.vector.tensor_reduce(out=nmax[i][:, j:j + 1], in_=pscore[:],
                                        axis=mybir.AxisListType.X,
                                        op=mybir.AluOpType.max, negate=True)
                nc.scalar.activation(out=expb[i][:, j * 512:(j + 1) * 512],
                                     in_=pscore[:],
                                     func=mybir.ActivationFunctionType.Exp,
                                     bias=nmax[i][:, j:j + 1], scale=1.0,
                                     accum_out=ssum[i][:, j:j + 1])
                if j == NB_T - 1:
                    finalize(i)

    nc.finalize()
    return nc


def _get_nc():
    if "nc" not in _CACHE:
        _CACHE["nc"] = _build()
    return _CACHE["nc"]


def kernel(out_state, history, W, b):
    from concourse.bass_utils import run_bass_kernel_spmd

    out_state = np.ascontiguousarray(out_state, dtype=np.float32)
    history = np.ascontiguousarray(history, dtype=np.float32)
    W = np.ascontiguousarray(W, dtype=np.float32)

    ht = np.ascontiguousarray(history.T)          # [N, S1]
    in_maps = []
    for c in range(NCORES):
        ost = np.ascontiguousarray(out_state[c * SC:(c + 1) * SC, :].T)  # [N, SC]
        in_maps.append({"w_r": W, "ost_r": ost, "ht_r": ht})

    nc = _get_nc()
    trace = bool(int(os.environ.get("KERNEL_TRACE", "0")))
    res = run_bass_kernel_spmd(nc, in_maps, list(range(NCORES)), trace=trace)
    _CACHE["last_result"] = res
    out = np.empty((S2, S1), dtype=np.float32)
    for c in range(NCORES):
        out[c * SC:(c + 1) * SC, :] = res.results[c]["probs"].astype(np.float32)
    return out


# revision 19
# speedup vs baseline: 1.0159x; 1.0159x over previous
"""TRN2 Bass kernel for nn_Attn_63230508532520.

reference:
    proj = history @ W.T + b            # [S1, N]
    energies = out_state @ proj.T       # [S2, S1]
    out = softmax(energies, axis=-1)

Math used here:
    energies = out_state @ W @ history.T + (out_state @ b) 1^T
    The bias term is constant per row -> softmax-invariant -> dropped.
    G = out_state @ W (per-core slice), scores = G @ history.T, row softmax.

Sharding: out_state rows (S2=4096) split across 8 cores (512 rows each);
W and history replicated. Host pre-transposes out_state slices and history
(pure layout choice), all matmul operands fed as float32r (TRN2 rounds to
1s8e11m internally; 4x faster than native fp32 matmul).

Per-core pipeline:
  Phase A: G.T [1024, 512] = W.T-stationary matmuls, accumulated over n.
  Phase B: for each t-block j (8 x 512 cols) stream history.T tiles, compute
           scores [128,512] per s-chunk i into PSUM, take block max (DVE),
           exp(x - blockmax) with per-row accumulation (ACT) into SBUF.
  Phase C: flash-style deferred normalization: global max per row from the 8
           block maxes, rescale factors exp(m_ij - M_i)/S_i, one multiply
           pass, DMA out.
"""
import os
import numpy as np
from contextlib import ExitStack

S2, S1, N = 4096, 4096, 1024
NCORES = 8
SC = S2 // NCORES          # 512 rows per core
NB_T = S1 // 512           # 8 t-blocks
NB_M = N // 128            # 8 contraction chunks
NB_I = SC // 128           # 4 s-chunks per core

_CACHE = {}


def _build():
    import concourse.bacc as bacc
    import concourse.mybir as mybir
    import concourse.tile as tile

    F32 = mybir.dt.float32
    F32R = mybir.dt.float32r

    nc = bacc.Bacc()
    w_r = nc.declare_dram_parameter("w_r", [N, N], F32R, isOutput=False)
    ost_r = nc.declare_dram_parameter("ost_r", [N, SC], F32R, isOutput=False)
    ht_r = nc.declare_dram_parameter("ht_r", [N, S1], F32R, isOutput=False)
    probs = nc.declare_dram_parameter("probs", [SC, S1], mybir.dt.float16, isOutput=True)

    with tile.TileContext(nc) as tc, ExitStack() as ctx:
        gt_pool = ctx.enter_context(tc.tile_pool(name="gt", bufs=1))
        exp_pool = ctx.enter_context(tc.tile_pool(name="exp", bufs=1))
        ht_pool = ctx.enter_context(tc.tile_pool(name="ht", bufs=2))
        small = ctx.enter_context(tc.tile_pool(name="small", bufs=1))
        ps = ctx.enter_context(tc.tile_pool(name="ps", bufs=8, space="PSUM"))

        # ---- Phase A: G.T = (out_state_slice @ W).T, [m, s] layout ----
        gt = []
        with tc.tile_pool(name="win", bufs=1) as win:
            w_sb, ost_sb = [], []
            for n in range(NB_M):
                wt = win.tile([128, N], F32R, tag=f"w{n}")
                nc.sync.dma_start(out=wt, in_=w_r[n * 128:(n + 1) * 128, :])
                w_sb.append(wt)
                ot = win.tile([128, SC], F32R, tag=f"o{n}")
                nc.sync.dma_start(out=ot, in_=ost_r[n * 128:(n + 1) * 128, :])
                ost_sb.append(ot)
            for m in range(NB_M):
                pg = ps.tile([128, SC], F32, tag="ps")
                for n in range(NB_M):
                    nc.tensor.matmul(pg[:], lhsT=w_sb[n][:, m * 128:(m + 1) * 128],
                                     rhs=ost_sb[n][:],
                                     start=(n == 0), stop=(n == NB_M - 1))
                g = gt_pool.tile([128, SC], F32R, tag=f"gt{m}")
                nc.vector.tensor_copy(out=g[:], in_=pg[:])
                gt.append(g)

        # ---- Phase B: scores + streaming exp ----
        out_pool = ctx.enter_context(tc.tile_pool(name="outp", bufs=2))
        expb = [exp_pool.tile([128, S1], mybir.dt.float16, tag=f"exp{i}", name=f"exp{i}") for i in range(NB_I)]
        nmax = [small.tile([128, NB_T], F32, tag=f"nmax{i}", name=f"nmax{i}") for i in range(NB_I)]
        ssum = [small.tile([128, NB_T], F32, tag=f"ssum{i}", name=f"ssum{i}") for i in range(NB_I)]

        def finalize(i):
            """Global max over block maxes, rescale factors, scale+store.

            nmax holds nm_ij = -m_ij; NM_i = min_j nm_ij = -M_i, so
            e_ij = exp(m_ij - M_i) = exp(-nm_ij + NM_i) = Exp(scale=-1, bias=NM_i).
            """
            nm = small.tile([128, 1], F32, tag=f"nm{i}", name=f"nm{i}")
            nc.vector.tensor_reduce(out=nm[:], in_=nmax[i][:],
                                    axis=mybir.AxisListType.X,
                                    op=mybir.AluOpType.min)
            e = small.tile([128, NB_T], F32, tag=f"e{i}", name=f"e{i}")
            nc.scalar.activation(out=e[:], in_=nmax[i][:],
                                 func=mybir.ActivationFunctionType.Exp,
                                 bias=nm[:], scale=-1.0)
            wsum = small.tile([128, NB_T], F32, tag=f"ws{i}", name=f"ws{i}")
            nc.vector.tensor_mul(wsum[:], e[:], ssum[i][:])
            s = small.tile([128, 1], F32, tag=f"s{i}", name=f"s{i}")
            nc.vector.tensor_reduce(out=s[:], in_=wsum[:],
                                    axis=mybir.AxisListType.X,
                                    op=mybir.AluOpType.add)
            r = small.tile([128, 1], F32, tag=f"r{i}", name=f"r{i}")
            nc.vector.reciprocal(out=r[:], in_=s[:])
            f = small.tile([128, NB_T], F32, tag=f"f{i}", name=f"f{i}")
            nc.vector.tensor_scalar_mul(f[:], e[:], r[:])
            o = out_pool.tile([128, S1], mybir.dt.float16,
                              tag=f"out{i % 2}", name=f"out{i}")
            for j in range(NB_T):
                sl = slice(j * 512, (j + 1) * 512)
                if j < 5:
                    nc.vector.tensor_scalar_mul(o[:, sl], expb[i][:, sl],
                                                f[:, j:j + 1])
                else:
                    nc.scalar.mul(o[:, sl], expb[i][:, sl], f[:, j:j + 1])
                if j == 3:
                    nc.sync.dma_start(out=probs[i * 128:(i + 1) * 128, 0:2048],
                                      in_=o[:, 0:2048])
            nc.scalar.dma_start(out=probs[i * 128:(i + 1) * 128, 2048:4096],
                                in_=o[:, 2048:4096])

        ht_quad = None
        for j in range(NB_T):
            if j % 2 == 0:
                ht_quad = []
                for m in range(NB_M):
                    t = ht_pool.tile([128, 1024], F32R, tag=f"ht{m}",
                                     name=f"ht{m}")
                    nc.sync.dma_start(out=t, in_=ht_r[m * 128:(m + 1) * 128,
                                                      j * 512:(j + 2) * 512])
                    ht_quad.append(t)
            off = (j % 2) * 512
            ht_sb = [t[:, off:off + 512] for t in ht_quad]
            for i in range(NB_I):
                pscore = ps.tile([128, 512], F32, tag="ps")
                for m in range(NB_M):
                    nc.tensor.matmul(pscore[:],
                                     lhsT=gt[m][:, i * 128:(i + 1) * 128],
                                     rhs=ht_sb[m][:],
                                     start=(m == 0), stop=(m == NB_M - 1))
                nc.vector.tensor_reduce(out=nmax[i][:, j:j + 1], in_=pscore[:],
                                        axis=mybir.AxisListType.X,
                                        op=mybir.AluOpType.max, negate=True)
                nc.scalar.activation(out=expb[i][:, j * 512:(j + 1) * 512],
                                     in_=pscore[:],
                                     func=mybir.ActivationFunctionType.Exp,
                                     bias=nmax[i][:, j:j + 1], scale=1.0,
                                     accum_out=ssum[i][:, j:j + 1])
                if j == NB_T - 1:
                    finalize(i)

    nc.finalize()
    return nc


def _get_nc():
    if "nc" not in _CACHE:
        _CACHE["nc"] = _build()
    return _CACHE["nc"]


def kernel(out_state, history, W, b):
    from concourse.bass_utils import run_bass_kernel_spmd

    out_state = np.ascontiguousarray(out_state, dtype=np.float32)
    history = np.ascontiguousarray(history, dtype=np.float32)
    W = np.ascontiguousarray(W, dtype=np.float32)

    ht = np.ascontiguousarray(history.T)          # [N, S1]
    in_maps = []
    for c in range(NCORES):
        ost = np.ascontiguousarray(out_state[c * SC:(c + 1) * SC, :].T)  # [N, SC]
        in_maps.append({"w_r": W, "ost_r": ost, "ht_r": ht})

    nc = _get_nc()
    trace = bool(int(os.environ.get("KERNEL_TRACE", "0")))
    res = run_bass_kernel_spmd(nc, in_maps, list(range(NCORES)), trace=trace)
    _CACHE["last_result"] = res
    out = np.empty((S2, S1), dtype=np.float32)
    for c in range(NCORES):
        out[c * SC:(c + 1) * SC, :] = res.results[c]["probs"].astype(np.float32)
    return out


# revision 32
# speedup vs baseline: 1.0229x; 1.0069x over previous
"""TRN2 Bass kernel for nn_Attn_63230508532520.

reference:
    proj = history @ W.T + b            # [S1, N]
    energies = out_state @ proj.T       # [S2, S1]
    out = softmax(energies, axis=-1)

Math used here:
    energies = out_state @ W @ history.T + (out_state @ b) 1^T
    The bias term is constant per row -> softmax-invariant -> dropped.
    G = out_state @ W (per-core slice), scores = G @ history.T, row softmax.

Sharding: out_state rows (S2=4096) split across 8 cores (512 rows each);
W and history replicated. Host pre-transposes out_state slices and history
(pure layout choice), all matmul operands fed as float32r (TRN2 rounds to
1s8e11m internally; 4x faster than native fp32 matmul).

Per-core pipeline (~107us HW exec; PE stream is within ~5% of its
68us fp32r roofline, HBM traffic 22MB in + 4MB out):
  Phase A: G.T [1024, 512] = W-stationary fp32r matmuls accumulated over n,
           PSUM evacuated to SBUF as float32r (the copy is the rounding).
  Phase B: for each t-block j (8 x 512 cols) stream history.T tiles
           ([128,1024] double-buffered loads), compute scores [128,512] per
           s-chunk i into a rotating PSUM bank (8 matmuls over m), take the
           block max (DVE reduce, negated for the exp bias), then
           exp(x - blockmax) straight out of PSUM with per-row accum_out
           (ACT) into an fp16 SBUF buffer (unnormalized exp <= 1).
  Phase C: flash-style deferred normalization, interleaved per s-chunk into
           the last t-block so it overlaps the remaining matmuls: global max
           from the 8 block maxes, factors f_ij = exp(m_ij - M_i)/S_i,
           scale slices split across DVE/ACT writing fp16 output tiles,
           stores split across both HW DGE rings. Output is fp16 (rounding
           5e-4, far below the fp32r matmul noise); host upcasts to fp32.
"""
import os
import numpy as np
from contextlib import ExitStack

S2, S1, N = 4096, 4096, 1024
NCORES = 8
SC = S2 // NCORES          # 512 rows per core
NB_T = S1 // 512           # 8 t-blocks
NB_M = N // 128            # 8 contraction chunks
NB_I = SC // 128           # 4 s-chunks per core

_CACHE = {}


def _build():
    import concourse.bacc as bacc
    import concourse.mybir as mybir
    import concourse.tile as tile

    F32 = mybir.dt.float32
    F32R = mybir.dt.float32r

    nc = bacc.Bacc()
    w_r = nc.declare_dram_parameter("w_r", [N, N], F32R, isOutput=False)
    ost_r = nc.declare_dram_parameter("ost_r", [N, SC], F32R, isOutput=False)
    ht_r = nc.declare_dram_parameter("ht_r", [N, S1], F32R, isOutput=False)
    probs = nc.declare_dram_parameter("probs", [SC, S1], mybir.dt.float16, isOutput=True)

    with tile.TileContext(nc) as tc, ExitStack() as ctx:
        gt_pool = ctx.enter_context(tc.tile_pool(name="gt", bufs=1))
        exp_pool = ctx.enter_context(tc.tile_pool(name="exp", bufs=1))
        ht_pool = ctx.enter_context(tc.tile_pool(name="ht", bufs=2))
        small = ctx.enter_context(tc.tile_pool(name="small", bufs=1))
        ps = ctx.enter_context(tc.tile_pool(name="ps", bufs=8, space="PSUM"))

        # ---- Phase A: G.T = (out_state_slice @ W).T, [m, s] layout ----
        gt = []
        with tc.tile_pool(name="win", bufs=1) as win:
            # w_r is host-rearranged panel-major: tile k holds W panel k
            # ([128 part, n*128+c] = W[n*128+p, k*128+c]), so G.T group m
            # depends only on panel m + ost, not on all of W.
            w_sb, ost_sb = [], []
            for n in range(NB_M):
                wt = win.tile([128, N], F32R, tag=f"w{n}")
                nc.sync.dma_start(out=wt, in_=w_r[n * 128:(n + 1) * 128, :])
                w_sb.append(wt)
                ot = win.tile([128, SC], F32R, tag=f"o{n}")
                nc.sync.dma_start(out=ot, in_=ost_r[n * 128:(n + 1) * 128, :])
                ost_sb.append(ot)
            for m in range(NB_M):
                pg = ps.tile([128, SC], F32, tag="ps")
                for n in range(NB_M):
                    nc.tensor.matmul(pg[:], lhsT=w_sb[m][:, n * 128:(n + 1) * 128],
                                     rhs=ost_sb[n][:],
                                     start=(n == 0), stop=(n == NB_M - 1))
                g = gt_pool.tile([128, SC], F32R, tag=f"gt{m}")
                nc.vector.tensor_copy(out=g[:], in_=pg[:])
                gt.append(g)

        # ---- Phase B: scores + streaming exp ----
        BLOCKS = [(k * 512, 512) for k in range(NB_T)]
        out_pool = ctx.enter_context(tc.tile_pool(name="outp", bufs=2))
        expb = [exp_pool.tile([128, S1], mybir.dt.float16, tag=f"exp{i}", name=f"exp{i}") for i in range(NB_I)]
        NBK = len(BLOCKS)
        nmax = [small.tile([128, NBK], F32, tag=f"nmax{i}", name=f"nmax{i}") for i in range(NB_I)]
        ssum = [small.tile([128, NBK], F32, tag=f"ssum{i}", name=f"ssum{i}") for i in range(NB_I)]

        def finalize(i):
            """Global max over block maxes, rescale factors, scale+store.

            nmax holds nm_ij = -m_ij; NM_i = min_j nm_ij = -M_i, so
            e_ij = exp(m_ij - M_i) = exp(-nm_ij + NM_i) = Exp(scale=-1, bias=NM_i).
            """
            nm = small.tile([128, 1], F32, tag=f"nm{i}", name=f"nm{i}")
            nc.vector.tensor_reduce(out=nm[:], in_=nmax[i][:],
                                    axis=mybir.AxisListType.X,
                                    op=mybir.AluOpType.min)
            e = small.tile([128, NBK], F32, tag=f"e{i}", name=f"e{i}")
            nc.scalar.activation(out=e[:], in_=nmax[i][:],
                                 func=mybir.ActivationFunctionType.Exp,
                                 bias=nm[:], scale=-1.0)
            wsum = small.tile([128, NBK], F32, tag=f"ws{i}", name=f"ws{i}")
            nc.vector.tensor_mul(wsum[:], e[:], ssum[i][:])
            s = small.tile([128, 1], F32, tag=f"s{i}", name=f"s{i}")
            nc.vector.tensor_reduce(out=s[:], in_=wsum[:],
                                    axis=mybir.AxisListType.X,
                                    op=mybir.AluOpType.add)
            r = small.tile([128, 1], F32, tag=f"r{i}", name=f"r{i}")
            nc.vector.reciprocal(out=r[:], in_=s[:])
            f = small.tile([128, NBK], F32, tag=f"f{i}", name=f"f{i}")
            nc.vector.tensor_scalar_mul(f[:], e[:], r[:])
            o = out_pool.tile([128, S1], mybir.dt.float16,
                              tag=f"out{i % 2}", name=f"out{i}")
            for b, (st, wd) in enumerate(BLOCKS):
                sl = slice(st, st + wd)
                if b < 5:
                    nc.vector.tensor_scalar_mul(o[:, sl], expb[i][:, sl],
                                                f[:, b:b + 1])
                else:
                    nc.scalar.mul(o[:, sl], expb[i][:, sl], f[:, b:b + 1])
                if b == 3:
                    nc.sync.dma_start(out=probs[i * 128:(i + 1) * 128, 0:2048],
                                      in_=o[:, 0:2048])
            nc.sync.dma_start(out=probs[i * 128:(i + 1) * 128, 2048:4096],
                               in_=o[:, 2048:4096])

        ht_pair = None
        cur_pair = -1
        for b, (st, wd) in enumerate(BLOCKS):
            pair = st // 1024
            if pair != cur_pair:
                cur_pair = pair
                ht_pair = []
                for m in range(NB_M):
                    t = ht_pool.tile([128, 1024], F32R, tag=f"ht{m}",
                                     name=f"ht{m}")
                    nc.sync.dma_start(out=t, in_=ht_r[m * 128:(m + 1) * 128,
                                                      pair * 1024:(pair + 1) * 1024])
                    ht_pair.append(t)
            off = st % 1024
            ht_sb = [t[:, off:off + wd] for t in ht_pair]
            for i in range(NB_I):
                pscore = ps.tile([128, 512], F32, tag="ps")
                for m in range(NB_M):
                    nc.tensor.matmul(pscore[:, 0:wd],
                                     lhsT=gt[m][:, i * 128:(i + 1) * 128],
                                     rhs=ht_sb[m][:],
                                     start=(m == 0), stop=(m == NB_M - 1))
                nc.vector.tensor_reduce(out=nmax[i][:, b:b + 1],
                                        in_=pscore[:, 0:wd],
                                        axis=mybir.AxisListType.X,
                                        op=mybir.AluOpType.max, negate=True)
                nc.scalar.activation(out=expb[i][:, st:st + wd],
                                     in_=pscore[:, 0:wd],
                                     func=mybir.ActivationFunctionType.Exp,
                                     bias=nmax[i][:, b:b + 1], scale=1.0,
                                     accum_out=ssum[i][:, b:b + 1])
                if b == len(BLOCKS) - 1:
                    finalize(i)

    nc.finalize()
    return nc


def _get_nc():
    if "nc" not in _CACHE:
        _CACHE["nc"] = _build()
    return _CACHE["nc"]


def kernel(out_state, history, W, b):
    from concourse.bass_utils import run_bass_kernel_spmd

    out_state = np.ascontiguousarray(out_state, dtype=np.float32)
    history = np.ascontiguousarray(history, dtype=np.float32)
    W = np.ascontiguousarray(W, dtype=np.float32)
    # panel-major layout: w_p[m*128+p, n*128+c] = W[n*128+p, m*128+c]
    W = np.ascontiguousarray(
        W.reshape(8, 128, 8, 128).transpose(2, 1, 0, 3).reshape(1024, 1024))

    ht = np.ascontiguousarray(history.T)          # [N, S1]
    in_maps = []
    for c in range(NCORES):
        ost = np.ascontiguousarray(out_state[c * SC:(c + 1) * SC, :].T)  # [N, SC]
        in_maps.append({"w_r": W, "ost_r": ost, "ht_r": ht})

    nc = _get_nc()
    trace = bool(int(os.environ.get("KERNEL_TRACE", "0")))
    res = run_bass_kernel_spmd(nc, in_maps, list(range(NCORES)), trace=trace)
    _CACHE["last_result"] = res
    out = np.empty((S2, S1), dtype=np.float32)
    for c in range(NCORES):
        out[c * SC:(c + 1) * SC, :] = res.results[c]["probs"].astype(np.float32)
    return out
